# revision 17
# baseline (speedup 1.0000x reference)
"""TRN2 Bass kernel for nn_CDF, v6: 64B gather blocks + fused iterations.

Per NeuronCore (8 cores, column-sharded 32 cols each):
  - blocked4[g16][s, 64]: superblock s = 4 sub-blocks [c4][rr][c]:
      blocked4[g16][s, 16c4 + 4rr + c] = order[4s + rr, 16g16 + 4c4 + c]
  - fused iteration = 512 batch rows x 16 cols (2 of v3's pair-groups):
    idx chain on [128, 64] tiles, one T1 + 8 T2 PE transposes build the
    wrapped idx layout for 8 gathers (c4 in [0,4) x rowhalf in [0,2)),
    each 1024 idxs with elem_size=16 f32 (64B), elem_step=64 (256B) --
    4x less gather traffic than 256B blocks.
  - select: stride-33 diagonal APs unified over c4 (4 DVE ops per
    128-row sub) + copy_predicated on idx&3.
  - outer loop over g16 so group-0 gathers start after half the prep.
"""

import numpy as np

import concourse.bacc as bacc
import concourse.bass as bass
import concourse.mybir as mybir
import concourse.tile as tile
from concourse import ap_utils
from concourse.bass_utils import run_bass_kernel_spmd
from concourse.masks import make_identity

N_CORES = 8
BATCH = 16384
N_DIM = 256
N_TRAIN = 100000
COLS = N_DIM // N_CORES          # 32 columns per core
P = 128

INV_SQRT2 = 0.7071067811865476

F32 = mybir.dt.float32
I32 = mybir.dt.int32
I16 = mybir.dt.int16
A = mybir.AluOpType


def raw_gather(nc, out_ap, in_ap, idxs_ap, num_idxs, elem_size, elem_step,
               queue_num):
    """dma_gather without the elem_size_bytes%256 assert (HW-validated:
    64B elem_size works when the stride is a multiple of 256B)."""
    eng = nc.gpsimd
    assert idxs_ap.dtype == mybir.dt.int16
    assert in_ap.dtype == out_ap.dtype
    assert ap_utils.ap_is_contiguous(out_ap.ap[1:])
    assert ap_utils.ap_is_contiguous(idxs_ap.ap[1:])
    assert in_ap.ap[-1][1] == out_ap.ap[-1][1] == elem_size
    assert in_ap.ap[0][0] == elem_step
    stride_bytes = elem_step * mybir.dt.size(in_ap.dtype)
    stride_bytes_256 = stride_bytes // 256
    assert stride_bytes % 256 == 0 and stride_bytes_256 < 256
    return eng.add_instruction(
        mybir.InstDMAGatherAnt(
            name=eng.bass.get_next_instruction_name(),
            ins=[*eng.lower_ap_dma(in_ap, for_custom_bir_dma=True),
                 eng.lower_ap(idxs_ap),
                 eng.lower_val_access(eng.to_reg(num_idxs))],
            outs=[eng.lower_ap(out_ap)],
            transpose=False,
            num_idxs=num_idxs,
            elem_size=elem_size,
            stride_bytes_256=stride_bytes_256,
            gen_mode=0,
            single_packet=True,
            queue_num=queue_num,
            sbuf_tokens_per_rank=0,
            sbuf_free_dim_per_rank=0,
            sbuf_free_dim_pad_per_rank=0,
            sbuf_byte_offset=0,
        )
    )


def _prep_group(nc, pool, order_d, blocked_d, n_train, n_groups, g):
    """order slice cols [16g,16g+16) -> blocked4[g][s, 64] = [c4][rr][c]."""
    rows_per_chunk = 4096
    r0 = 0
    while r0 < n_train:
        rows = min(rows_per_chunk, n_train - r0)
        parts = rows // 32
        assert rows % 32 == 0
        stage = pool.tile([P, 512], F32, tag="prep")
        # stage[p, 16*rr32 + cc] = order[r0 + 32p + rr32, 16g + cc]
        src = order_d.ap()
        src_ap = bass.AP(
            src.tensor,
            src.offset + r0 * (16 * n_groups) + 16 * g,
            [[32 * 16 * n_groups, parts], [16 * n_groups, 32], [1, 16]],
        )
        nc.scalar.dma_start(
            stage[:parts, :].rearrange("p (rr c) -> p rr c", c=16), src_ap)
        # permute: stage2[p, 64rr8+16c4+4rr+c] = stage[p, 64rr8+16rr+4c4+c]
        stage2 = pool.tile([P, 512], F32, tag="prep2")
        sv, dv = stage[:parts, :], stage2[:parts, :]
        for rr in range(4):
            src_p = bass.AP(sv.tensor, sv.offset + 16 * rr,
                            [list(sv.ap[0]), [64, 8], [4, 4], [1, 4]])
            dst_p = bass.AP(dv.tensor, dv.offset + 4 * rr,
                            [list(dv.ap[0]), [64, 8], [16, 4], [1, 4]])
            nc.vector.tensor_copy(dst_p, src_p)
        # blocked4[g][r0//4 + 8p + rr8, :] = stage2[p, 64rr8 : +64]
        dst = blocked_d.ap()
        dst_ap = bass.AP(
            dst.tensor,
            dst.offset + (r0 // 4) * 64,
            [[8 * 64, parts], [64, 8], [1, 64]],
        )
        nc.scalar.dma_start(
            dst_ap,
            stage2[:parts, :].rearrange("p (rr2 x) -> p rr2 x", x=64))
        r0 += rows


def build_nc(batch=BATCH, n_train=N_TRAIN, cols=COLS, nq=4,
             act_fn=None):
    assert n_train % 4 == 0
    n_groups = cols // 16
    ns = n_train // 4
    assert ns - 1 <= 32767
    n_fused = batch // 512          # 512 rows per fused iteration

    nc = bacc.Bacc("TRN2", target_bir_lowering=False, debug=False,
                   num_swdge_queues=nq, dynamic_dma_scratch_size=2 ** 16)
    noise_d = nc.dram_tensor("noise", [batch, cols], F32, kind="ExternalInput")
    order_d = nc.dram_tensor("order", [n_train, cols], F32,
                             kind="ExternalInput")
    out_d = nc.dram_tensor("out", [batch, cols], F32, kind="ExternalOutput")
    blocked_ds = [
        nc.dram_tensor(f"blocked{g}", [ns, 64], F32, kind="Internal")
        for g in range(n_groups)
    ]

    gq = [0]

    with tile.TileContext(nc) as tc:
        with tc.tile_pool(name="const", bufs=1) as cpool, \
             tc.tile_pool(name="prep", bufs=8) as ppool, \
             tc.tile_pool(name="work", bufs=8) as wpool, \
             tc.tile_pool(name="idxp", bufs=10) as ipool, \
             tc.tile_pool(name="gath", bufs=20) as gpool, \
             tc.tile_pool(name="psum", bufs=4, space="PSUM") as pspool:

            # lw[w][i, p] = 1.0 iff i == 16w + p%16  (stationary selector:
            # one matmul both transposes AND replicates the wrapped idxs)
            ci = cpool.tile([P, P], I32)
            nc.gpsimd.iota(ci[:], [[0, P]], base=0, channel_multiplier=1)
            pf = cpool.tile([P, P], I32)
            nc.gpsimd.iota(pf[:], [[1, P]], base=0, channel_multiplier=0)
            cil = cpool.tile([P, P], I32)
            nc.vector.tensor_scalar(cil[:], ci[:], 15, None, A.bitwise_and)
            nc.vector.tensor_scalar(pf[:], pf[:], 15, None, A.bitwise_and)
            t16i = cpool.tile([P, P], I32)
            nc.vector.tensor_tensor(t16i[:], cil[:], pf[:], A.is_equal)
            cih = cpool.tile([P, P], I32)
            nc.vector.tensor_scalar(cih[:], ci[:], 4, None,
                                    A.arith_shift_right)
            lws = []
            for w in range(8):
                mw = cpool.tile([P, P], I32, tag=f"mw{w}")
                nc.vector.tensor_scalar(mw[:], cih[:], w, None, A.is_equal)
                nc.vector.tensor_tensor(mw[:], mw[:], t16i[:], A.bitwise_and)
                lw = cpool.tile([P, P], F32, tag=f"lw{w}")
                nc.vector.tensor_copy(lw[:], mw[:])
                lws.append(lw)

            def phase_a(pp, g16):
                i0 = pp * 512
                # ---- load noise [128, 64]: [p, 16*sub + cc], sub in [0,4)
                x = wpool.tile([P, 64], F32, tag="x")
                nap = noise_d.ap()
                src_ap = bass.AP(
                    nap.tensor, nap.offset + i0 * cols + 16 * g16,
                    [[cols, P], [P * cols, 4], [1, 16]],
                )
                nc.scalar.dma_start(
                    x[:].rearrange("p (s c) -> p s c", c=16), src_ap)

                # ---- index chain on [128, 64] ----
                e = wpool.tile([P, 64], F32, tag="e")
                nc.scalar.activation(e[:], x[:],
                                     act_fn or mybir.ActivationFunctionType.Erf,
                                     scale=INV_SQRT2)
                tf = wpool.tile([P, 64], F32, tag="tf")
                nc.vector.tensor_scalar(tf[:], e[:], 0.5 * n_train,
                                        0.5 * n_train - 0.5, A.mult, A.add)
                ti = wpool.tile([P, 64], I32, tag="ti")
                nc.vector.tensor_copy(ti[:], tf[:])
                nc.vector.tensor_scalar(ti[:], ti[:], n_train - 1, 0,
                                        A.min, A.max)
                lo2 = wpool.tile([P, 64], I32, tag="lo2")
                nc.vector.tensor_scalar(lo2[:], ti[:], 3, None, A.bitwise_and)
                s32 = wpool.tile([P, 64], I32, tag="s32")
                nc.vector.tensor_scalar(s32[:], ti[:], 2, None,
                                        A.arith_shift_right)
                sf = wpool.tile([P, 64], F32, tag="sf")
                nc.vector.tensor_copy(sf[:], s32[:])

                # ---- per w: rep[p, cc'] = sf[16w + p%16, cc'] via one
                # matmul with the banded selector (transpose + 8-way
                # replication fused); then 2 copies into the wrapped idxs
                # layout idxs[p, 128c4 + 64rh + 16c + 8s2 + w]
                idxs = ipool.tile([P, 512], I16, tag="idxs")
                for w in range(8):
                    rep_ps = pspool.tile([P, 64], F32, tag="t2")
                    nc.tensor.matmul(rep_ps[:], lws[w][:], sf[:],
                                     start=True, stop=True)
                    iap = idxs[:]
                    for rh in range(2):
                        dst = bass.AP(iap.tensor, iap.offset + w + 64 * rh,
                                      [[512, P], [8, 2], [128, 4], [16, 4]])
                        src = rep_ps[:, 32 * rh:32 * rh + 32].rearrange(
                            "a (s2 c4 c) -> a s2 c4 c", s2=2, c4=4)
                        nc.scalar.activation(
                            dst, src, mybir.ActivationFunctionType.Copy)

                # ---- masks (shared) ----
                masks = []
                for r in range(1, 4):
                    m = wpool.tile([P, 64], I32, tag=f"m{r}")
                    nc.vector.tensor_scalar(m[:], lo2[:], r, None, A.is_equal)
                    masks.append(m)

                # ---- 8 gathers (c4 x rowhalf), 64B elems ----
                bap = blocked_ds[g16].ap()
                gs = []
                for rh in range(2):
                    g = gpool.tile([P, 512], F32, tag=f"g{rh}")
                    for c4 in range(4):
                        j = 2 * c4 + rh
                        in_ap = bass.AP(bap.tensor, bap.offset + 16 * c4,
                                        [[64, ns], [1, 16]])
                        raw_gather(
                            nc,
                            g[:, 128 * c4:128 * c4 + 128].rearrange(
                                "p (n x) -> p n x", x=16),
                            in_ap,
                            idxs[:, 64 * j:64 * j + 64],
                            1024, 16, 64, gq[0] % nq)
                        gq[0] += 1
                    gs.append(g)
                return pp, g16, gs, masks

            def phase_b(state):
                pp, g16, gs, masks = state
                i0 = pp * 512
                acc = wpool.tile([P, 64], F32, tag="acc")
                for rh in range(2):
                    gv = gs[rh][:]
                    for s2 in range(2):
                        sub = 2 * rh + s2
                        # elem = 128c4 + 33c + 16s2 + 4r
                        def cand(r):
                            return bass.AP(gv.tensor,
                                           gv.offset + 16 * s2 + 4 * r,
                                           [list(gv.ap[0]), [33, 4],
                                            [128, 4]])

                        lo = 16 * sub
                        av = acc[:]
                        acc_sl = bass.AP(av.tensor, av.offset + lo,
                                         [list(av.ap[0]), [1, 4], [4, 4]])
                        msl = []
                        for r in range(1, 4):
                            mv = masks[r - 1][:]
                            msl.append(bass.AP(mv.tensor, mv.offset + lo,
                                               [list(mv.ap[0]), [1, 4],
                                                [4, 4]]))
                        nc.vector.tensor_copy(acc_sl, cand(0))
                        for r in range(1, 4):
                            nc.vector.copy_predicated(acc_sl, msl[r - 1],
                                                      cand(r))

                # ---- write out [128, 64] -> out rows ----
                oap = out_d.ap()
                dst_ap = bass.AP(
                    oap.tensor, oap.offset + i0 * cols + 16 * g16,
                    [[cols, P], [P * cols, 4], [1, 16]],
                )
                nc.sync.dma_start(
                    dst_ap, acc[:].rearrange("p (s c) -> p s c", c=16))

            # software pipeline: selects of iter k run after gathers of
            # iters k+1 AND k+2 have been issued (skew 2), so the DVE's
            # in-order select wait never starves the gather queues.
            pending = []
            for g16 in range(n_groups):
                _prep_group(nc, ppool, order_d, blocked_ds[g16], n_train,
                            n_groups, g16)
                for pp in range(n_fused):
                    pending.append(phase_a(pp, g16))
                    if len(pending) > 4:
                        phase_b(pending.pop(0))
            for st in pending:
                phase_b(st)

    nc.compile()
    return nc


_nc_cache = {}


def _get_nc():
    if "nc" not in _nc_cache:
        _nc_cache["nc"] = build_nc()
    return _nc_cache["nc"]


def kernel(noise: np.ndarray, order: np.ndarray) -> np.ndarray:
    noise = np.ascontiguousarray(np.asarray(noise, dtype=np.float32))
    order = np.ascontiguousarray(np.asarray(order, dtype=np.float32))
    assert noise.shape == (BATCH, N_DIM)
    assert order.shape == (N_TRAIN, N_DIM)
    nc = _get_nc()
    in_maps = [
        {
            "noise": np.ascontiguousarray(noise[:, c * COLS:(c + 1) * COLS]),
            "order": np.ascontiguousarray(order[:, c * COLS:(c + 1) * COLS]),
        }
        for c in range(N_CORES)
    ]
    res = run_bass_kernel_spmd(nc, in_maps, core_ids=list(range(N_CORES)))
    return np.concatenate([r["out"] for r in res.results], axis=1)


# revision 21
# speedup vs baseline: 1.0288x; 1.0288x over previous
"""TRN2 Bass kernel for nn_CDF, v6: 64B gather blocks + fused iterations.

Per NeuronCore (8 cores, column-sharded 32 cols each):
  - blocked4[g16][s, 64]: superblock s = 4 sub-blocks [c4][rr][c]:
      blocked4[g16][s, 16c4 + 4rr + c] = order[4s + rr, 16g16 + 4c4 + c]
  - fused iteration = 512 batch rows x 16 cols (2 of v3's pair-groups):
    idx chain on [128, 64] tiles, one T1 + 8 T2 PE transposes build the
    wrapped idx layout for 8 gathers (c4 in [0,4) x rowhalf in [0,2)),
    each 1024 idxs with elem_size=16 f32 (64B), elem_step=64 (256B) --
    4x less gather traffic than 256B blocks.
  - select: stride-33 diagonal APs unified over c4 (4 DVE ops per
    128-row sub) + copy_predicated on idx&3.
  - outer loop over g16 so group-0 gathers start after half the prep.
"""

import numpy as np

import concourse.bacc as bacc
import concourse.bass as bass
import concourse.mybir as mybir
import concourse.tile as tile
from concourse import ap_utils
from concourse.bass_utils import run_bass_kernel_spmd
from concourse.masks import make_identity

N_CORES = 8
BATCH = 16384
N_DIM = 256
N_TRAIN = 100000
COLS = N_DIM // N_CORES          # 32 columns per core
P = 128

INV_SQRT2 = 0.7071067811865476

F32 = mybir.dt.float32
I32 = mybir.dt.int32
I16 = mybir.dt.int16
A = mybir.AluOpType


def raw_gather(nc, out_ap, in_ap, idxs_ap, num_idxs, elem_size, elem_step,
               queue_num, single_packet=True):
    """dma_gather without the elem_size_bytes%256 assert (HW-validated:
    64B elem_size works when the stride is a multiple of 256B)."""
    eng = nc.gpsimd
    assert idxs_ap.dtype == mybir.dt.int16
    assert in_ap.dtype == out_ap.dtype
    assert ap_utils.ap_is_contiguous(out_ap.ap[1:])
    assert ap_utils.ap_is_contiguous(idxs_ap.ap[1:])
    assert in_ap.ap[-1][1] == out_ap.ap[-1][1] == elem_size
    assert in_ap.ap[0][0] == elem_step
    stride_bytes = elem_step * mybir.dt.size(in_ap.dtype)
    stride_bytes_256 = stride_bytes // 256
    assert stride_bytes % 256 == 0 and stride_bytes_256 < 256
    return eng.add_instruction(
        mybir.InstDMAGatherAnt(
            name=eng.bass.get_next_instruction_name(),
            ins=[*eng.lower_ap_dma(in_ap, for_custom_bir_dma=True),
                 eng.lower_ap(idxs_ap),
                 eng.lower_val_access(eng.to_reg(num_idxs))],
            outs=[eng.lower_ap(out_ap)],
            transpose=False,
            num_idxs=num_idxs,
            elem_size=elem_size,
            stride_bytes_256=stride_bytes_256,
            gen_mode=0,
            single_packet=single_packet,
            queue_num=queue_num,
            sbuf_tokens_per_rank=0,
            sbuf_free_dim_per_rank=0,
            sbuf_free_dim_pad_per_rank=0,
            sbuf_byte_offset=0,
        )
    )


def _prep_group(nc, pool, order_d, blocked_d, n_train, n_groups, g):
    """order slice cols [16g,16g+16) -> blocked4[g][s, 64] = [c4][rr][c]."""
    rows_per_chunk = 4096
    r0 = 0
    while r0 < n_train:
        rows = min(rows_per_chunk, n_train - r0)
        parts = rows // 32
        assert rows % 32 == 0
        stage = pool.tile([P, 512], F32, tag="prep")
        # stage[p, 16*rr32 + cc] = order[r0 + 32p + rr32, 16g + cc]
        src = order_d.ap()
        src_ap = bass.AP(
            src.tensor,
            src.offset + r0 * (16 * n_groups) + 16 * g,
            [[32 * 16 * n_groups, parts], [16 * n_groups, 32], [1, 16]],
        )
        nc.scalar.dma_start(
            stage[:parts, :].rearrange("p (rr c) -> p rr c", c=16), src_ap)
        # permute: stage2[p, 64rr8+16c4+4rr+c] = stage[p, 64rr8+16rr+4c4+c]
        stage2 = pool.tile([P, 512], F32, tag="prep2")
        sv, dv = stage[:parts, :], stage2[:parts, :]
        for rr in range(4):
            src_p = bass.AP(sv.tensor, sv.offset + 16 * rr,
                            [list(sv.ap[0]), [64, 8], [4, 4], [1, 4]])
            dst_p = bass.AP(dv.tensor, dv.offset + 4 * rr,
                            [list(dv.ap[0]), [64, 8], [16, 4], [1, 4]])
            nc.vector.tensor_copy(dst_p, src_p)
        # blocked4[g][r0//4 + 8p + rr8, :] = stage2[p, 64rr8 : +64]
        dst = blocked_d.ap()
        dst_ap = bass.AP(
            dst.tensor,
            dst.offset + (r0 // 4) * 64,
            [[8 * 64, parts], [64, 8], [1, 64]],
        )
        nc.scalar.dma_start(
            dst_ap,
            stage2[:parts, :].rearrange("p (rr2 x) -> p rr2 x", x=64))
        r0 += rows


def build_nc(batch=BATCH, n_train=N_TRAIN, cols=COLS, nq=4,
             act_fn=None):
    assert n_train % 4 == 0
    n_groups = cols // 16
    ns = n_train // 4
    assert ns - 1 <= 32767
    n_fused = batch // 512          # 512 rows per fused iteration

    nc = bacc.Bacc("TRN2", target_bir_lowering=False, debug=False,
                   num_swdge_queues=nq, dynamic_dma_scratch_size=2 ** 16)
    noise_d = nc.dram_tensor("noise", [batch, cols], F32, kind="ExternalInput")
    order_d = nc.dram_tensor("order", [n_train, cols], F32,
                             kind="ExternalInput")
    out_d = nc.dram_tensor("out", [batch, cols], F32, kind="ExternalOutput")
    blocked_ds = [
        nc.dram_tensor(f"blocked{g}", [ns, 64], F32, kind="Internal")
        for g in range(n_groups)
    ]

    gq = [0]

    with tile.TileContext(nc) as tc:
        with tc.tile_pool(name="const", bufs=1) as cpool, \
             tc.tile_pool(name="prep", bufs=4) as ppool, \
             tc.tile_pool(name="work", bufs=8) as wpool, \
             tc.tile_pool(name="idxp", bufs=10) as ipool, \
             tc.tile_pool(name="gath", bufs=12) as gpool, \
             tc.tile_pool(name="psum", bufs=4, space="PSUM") as pspool:

            # lw[w][i, p] = 1.0 iff i == 16w + p%16  (stationary selector:
            # one matmul both transposes AND replicates the wrapped idxs)
            ci = cpool.tile([P, P], I32)
            nc.gpsimd.iota(ci[:], [[0, P]], base=0, channel_multiplier=1)
            pf = cpool.tile([P, P], I32)
            nc.gpsimd.iota(pf[:], [[1, P]], base=0, channel_multiplier=0)
            cil = cpool.tile([P, P], I32)
            nc.vector.tensor_scalar(cil[:], ci[:], 15, None, A.bitwise_and)
            nc.vector.tensor_scalar(pf[:], pf[:], 15, None, A.bitwise_and)
            t16i = cpool.tile([P, P], I32)
            nc.vector.tensor_tensor(t16i[:], cil[:], pf[:], A.is_equal)
            cih = cpool.tile([P, P], I32)
            nc.vector.tensor_scalar(cih[:], ci[:], 4, None,
                                    A.arith_shift_right)
            lws = []
            for w in range(8):
                mw = cpool.tile([P, P], I32, tag=f"mw{w}")
                nc.vector.tensor_scalar(mw[:], cih[:], w, None, A.is_equal)
                nc.vector.tensor_tensor(mw[:], mw[:], t16i[:], A.bitwise_and)
                lw = cpool.tile([P, P], F32, tag=f"lw{w}")
                nc.vector.tensor_copy(lw[:], mw[:])
                lws.append(lw)

            def phase_a(pp, g16):
                i0 = pp * 512
                # ---- load noise [128, 64]: [p, 16*sub + cc], sub in [0,4)
                x = wpool.tile([P, 64], F32, tag="x")
                nap = noise_d.ap()
                src_ap = bass.AP(
                    nap.tensor, nap.offset + i0 * cols + 16 * g16,
                    [[cols, P], [P * cols, 4], [1, 16]],
                )
                nc.scalar.dma_start(
                    x[:].rearrange("p (s c) -> p s c", c=16), src_ap)

                # ---- index chain on [128, 64] ----
                e = wpool.tile([P, 64], F32, tag="e")
                nc.scalar.activation(e[:], x[:],
                                     act_fn or mybir.ActivationFunctionType.Erf,
                                     scale=INV_SQRT2)
                tf = wpool.tile([P, 64], F32, tag="tf")
                nc.vector.tensor_scalar(tf[:], e[:], 0.5 * n_train,
                                        0.5 * n_train - 0.5, A.mult, A.add)
                ti = wpool.tile([P, 64], I32, tag="ti")
                nc.vector.tensor_copy(ti[:], tf[:])
                nc.vector.tensor_scalar(ti[:], ti[:], n_train - 1, 0,
                                        A.min, A.max)
                lo2 = wpool.tile([P, 64], I32, tag="lo2")
                nc.vector.tensor_scalar(lo2[:], ti[:], 3, None, A.bitwise_and)
                s32 = wpool.tile([P, 64], I32, tag="s32")
                nc.vector.tensor_scalar(s32[:], ti[:], 2, None,
                                        A.arith_shift_right)
                sf = wpool.tile([P, 64], F32, tag="sf")
                nc.vector.tensor_copy(sf[:], s32[:])

                # ---- per w: rep[p, cc'] = sf[16w + p%16, cc'] via one
                # matmul with the banded selector (transpose + 8-way
                # replication fused); then 2 copies into the wrapped idxs
                # layout idxs[p, 128c4 + 64rh + 16c + 8s2 + w]
                idxs = ipool.tile([P, 512], I16, tag="idxs")
                for w in range(8):
                    rep_ps = pspool.tile([P, 64], F32, tag="t2")
                    nc.tensor.matmul(rep_ps[:], lws[w][:], sf[:],
                                     start=True, stop=True)
                    iap = idxs[:]
                    for rh in range(2):
                        dst = bass.AP(iap.tensor, iap.offset + w + 64 * rh,
                                      [[512, P], [8, 2], [128, 4], [16, 4]])
                        src = rep_ps[:, 32 * rh:32 * rh + 32].rearrange(
                            "a (s2 c4 c) -> a s2 c4 c", s2=2, c4=4)
                        nc.scalar.activation(
                            dst, src, mybir.ActivationFunctionType.Copy)

                # ---- masks (shared) ----
                masks = []
                for r in range(1, 4):
                    m = wpool.tile([P, 64], I32, tag=f"m{r}")
                    nc.vector.tensor_scalar(m[:], lo2[:], r, None, A.is_equal)
                    masks.append(m)

                # ---- 4 gathers (one per c4, both rowhalves merged:
                # 2048 idxs, single_packet=False), 64B elems ----
                bap = blocked_ds[g16].ap()
                gs = []
                for c4 in range(4):
                    g = gpool.tile([P, 256], F32, tag=f"g{c4}")
                    in_ap = bass.AP(bap.tensor, bap.offset + 16 * c4,
                                    [[64, ns], [1, 16]])
                    raw_gather(
                        nc,
                        g[:].rearrange("p (n x) -> p n x", x=16),
                        in_ap,
                        idxs[:, 128 * c4:128 * c4 + 128],
                        2048, 16, 64, gq[0] % nq, single_packet=False)
                    gq[0] += 1
                    gs.append(g)
                return pp, g16, gs, masks

            def phase_b(state):
                pp, g16, gs, masks = state
                i0 = pp * 512
                acc = wpool.tile([P, 64], F32, tag="acc")
                for c4 in range(4):
                    gv = gs[c4][:]
                    for s2 in range(2):
                        # gathered elem = 128rh + 33c + 16s2 + 4r
                        def cand(r):
                            return bass.AP(gv.tensor,
                                           gv.offset + 16 * s2 + 4 * r,
                                           [list(gv.ap[0]), [128, 2],
                                            [33, 4]])

                        # acc[p, 16*(2rh + s2) + 4c4 + c], iterate (rh, c)
                        lo = 16 * s2 + 4 * c4
                        av = acc[:]
                        acc_sl = bass.AP(av.tensor, av.offset + lo,
                                         [list(av.ap[0]), [32, 2], [1, 4]])
                        msl = []
                        for r in range(1, 4):
                            mv = masks[r - 1][:]
                            msl.append(bass.AP(mv.tensor, mv.offset + lo,
                                               [list(mv.ap[0]), [32, 2],
                                                [1, 4]]))
                        nc.vector.tensor_copy(acc_sl, cand(0))
                        for r in range(1, 4):
                            nc.vector.copy_predicated(acc_sl, msl[r - 1],
                                                      cand(r))

                # ---- write out [128, 64] -> out rows ----
                oap = out_d.ap()
                dst_ap = bass.AP(
                    oap.tensor, oap.offset + i0 * cols + 16 * g16,
                    [[cols, P], [P * cols, 4], [1, 16]],
                )
                nc.sync.dma_start(
                    dst_ap, acc[:].rearrange("p (s c) -> p s c", c=16))

            # software pipeline: selects of iter k run after gathers of
            # iters k+1 AND k+2 have been issued (skew 2), so the DVE's
            # in-order select wait never starves the gather queues.
            pending = []
            for g16 in range(n_groups):
                _prep_group(nc, ppool, order_d, blocked_ds[g16], n_train,
                            n_groups, g16)
                for pp in range(n_fused):
                    pending.append(phase_a(pp, g16))
                    if len(pending) > 4:
                        phase_b(pending.pop(0))
            for st in pending:
                phase_b(st)

    nc.compile()
    return nc


_nc_cache = {}


def _get_nc():
    if "nc" not in _nc_cache:
        _nc_cache["nc"] = build_nc()
    return _nc_cache["nc"]


def kernel(noise: np.ndarray, order: np.ndarray) -> np.ndarray:
    noise = np.ascontiguousarray(np.asarray(noise, dtype=np.float32))
    order = np.ascontiguousarray(np.asarray(order, dtype=np.float32))
    assert noise.shape == (BATCH, N_DIM)
    assert order.shape == (N_TRAIN, N_DIM)
    nc = _get_nc()
    in_maps = [
        {
            "noise": np.ascontiguousarray(noise[:, c * COLS:(c + 1) * COLS]),
            "order": np.ascontiguousarray(order[:, c * COLS:(c + 1) * COLS]),
        }
        for c in range(N_CORES)
    ]
    res = run_bass_kernel_spmd(nc, in_maps, core_ids=list(range(N_CORES)))
    return np.concatenate([r["out"] for r in res.results], axis=1)
